# revision 1
# baseline (speedup 1.0000x reference)
"""CTLSTM Trainium2 kernel, v2: sorted static expansion.

The reference re-inits h/c/c_bar to zero every step, so all gate values
depend ONLY on the event type (1001 distinct embedding rows). v1 built a
per-type gate table in DRAM and DMA-gathered per token; the gather's SWDGE
descriptor emission (10.6us/1024 tokens on the Q7) and its HBM reads
(33MB/core) dominated.

v2 removes the gather entirely:
  - Host sorts tokens by event type and renumbers types in count-sorted
    order, so equal-count types are contiguous and each token wave reads a
    contiguous, non-decreasing range of table columns.
  - Device builds the gate tables TRANSPOSED in SBUF as one interleaved
    tile [128 gate-cols, 4 slabs (c|cb|go|gd), 1024 types], via matmul with
    the weight tile stationary and embedding columns streaming; bias applied
    by the ACT bias-vector port.
  - Per-token expansion = a few STATIC stride-0 broadcast copies per wave
    (one per repeat-count bucket, all 4 slabs at once), split across
    ACT/DVE/GPSIMD by a greedy cost balance. No indices, no per-token
    descriptors, no HBM reads.
  - h_d = go*tanh(cb + (c-cb)*exp(-gd*dur)) in the same transposed layout;
    sorted dur broadcast across partitions via a PE ones-outer-product and
    consumed directly from PSUM.
  - Outputs stream out densely in sorted order; the host un-permutes.

Sharding: each core owns a 128-wide slice of H for all 5 gate groups and
processes all 16384 tokens (outputs are disjoint in H).
"""

import os

import numpy as np

HIDDEN = 1024
TYPES = 1001
TPAD = 1024
B = 32
T = 512
NTOK = B * T          # 16384
NCORES = 8
NGATES = 5            # i, z, o, ibar, d (f, fbar unused by the reference)
GATE_ROWS = (0, 2, 3, 4, 6)
KT = HIDDEN // 128    # 8 contraction tiles
TCHUNKS = ((0, 128), (128, 384), (512, 512))  # type-chunk plan
CSPLIT = 640          # chain column split: DVE [0:CSPLIT), GPS [CSPLIT:WAVE)
WAVE = 1024           # tokens per phase-2 wave
NWAVES = NTOK // WAVE

LAST_RESULTS = None
_CACHED = None        # (ev_bytes_hash, nc)


def _make_pieces(ev_tok):
    """Host-side: count-sort types, sort tokens, and cut the per-type runs
    into per-wave broadcast pieces.

    Returns (type_order, perm, pieces_by_wave) where each piece is
    (t0, ntypes, reps, j0): expanded[:, j0 : j0+ntypes*reps] =
    repeat(table[:, t0:t0+ntypes], reps).
    """
    counts = np.bincount(ev_tok, minlength=TYPES)
    type_order = np.argsort(-counts, kind="stable")         # descending count
    new_id = np.empty(TYPES, np.int64)
    new_id[type_order] = np.arange(TYPES)
    key = new_id[ev_tok]
    perm = np.argsort(key, kind="stable")
    counts_sorted = counts[type_order]

    pieces_by_wave = [[] for _ in range(NWAVES)]
    j = 0
    raw = []  # (t, j, reps) per (type, wave) fragment
    for t in range(TYPES):
        c = int(counts_sorted[t])
        if c == 0:
            continue
        left = c
        while left > 0:
            w = j // WAVE
            take = min(left, (w + 1) * WAVE - j)
            raw.append((t, j, take))
            j += take
            left -= take
    assert j == NTOK
    # merge consecutive same-rep, consecutive-type fragments within a wave
    for t, j0, reps in raw:
        w = j0 // WAVE
        lst = pieces_by_wave[w]
        if lst and lst[-1][2] == reps and lst[-1][0] + lst[-1][1] == t \
                and lst[-1][3] + lst[-1][1] * reps == j0:
            t0, nt, r, jj = lst[-1]
            lst[-1] = (t0, nt + 1, r, jj)
        else:
            lst.append((t, 1, reps, j0))
    return type_order, perm, pieces_by_wave


def _build_nc(pieces_by_wave):
    import concourse.mybir as mybir
    from concourse import bacc
    from concourse.tile import TileContext

    dt = mybir.dt
    AF = mybir.ActivationFunctionType
    f32 = dt.float32
    bf16 = dt.bfloat16

    nc = bacc.Bacc("TRN2", target_bir_lowering=False, debug=False)

    et_d = nc.dram_tensor("et", [HIDDEN, TPAD], bf16, kind="ExternalInput")
    wt_d = nc.dram_tensor("wt", [HIDDEN, NGATES * 128], bf16, kind="ExternalInput")
    bias_d = nc.dram_tensor("bias", [128, NGATES], f32, kind="ExternalInput")
    dur_d = nc.dram_tensor("durneg", [1, NTOK], bf16, kind="ExternalInput")
    out_d = nc.dram_tensor("out", [5, 128, NTOK], f32, kind="ExternalOutput")

    with TileContext(nc) as tc:
        with (
            tc.tile_pool(name="const", bufs=1) as cpool,
            tc.tile_pool(name="p1ps", bufs=2, space="PSUM") as p1ps,
            tc.tile_pool(name="p2ps", bufs=4, space="PSUM") as p2ps,
            tc.tile_pool(name="p1t", bufs=2) as p1t,
            tc.tile_pool(name="wave", bufs=3) as wpool,
        ):
            # ---- constant loads ------------------------------------------------
            wt_sb = cpool.tile([128, KT, NGATES * 128], bf16, tag="wt")
            nc.sync.dma_start(
                out=wt_sb[:], in_=wt_d[:].rearrange("(kt p) n -> p kt n", p=128))
            et_sb = cpool.tile([128, KT, TPAD], bf16, tag="et")
            et_r = et_d[:].rearrange("(kt p) t -> p kt t", p=128)
            for c0, cn in TCHUNKS:
                ts = slice(c0, c0 + cn)
                nc.sync.dma_start(out=et_sb[:, :, ts], in_=et_r[:, :, ts])
            bias_sb = cpool.tile([128, NGATES], f32, tag="bias")
            nc.sync.dma_start(out=bias_sb[:], in_=bias_d[:])
            dur_sb = cpool.tile([1, NTOK], bf16, tag="dur")
            nc.sync.dma_start(out=dur_sb[:], in_=dur_d[:])
            ones_sb = cpool.tile([1, 128], bf16, tag="ones")
            nc.vector.memset(ones_sb[:], 1.0)

            # ---- phase 1: interleaved gate table [128, 4(c|cb|go|gd), TPAD] ----
            tab = cpool.tile([128, 4, TPAD], f32, tag="tab")

            for c0, cn in TCHUNKS:
                ts = slice(c0, c0 + cn)
                # emission order groups the sigmoids to limit ACT table swaps:
                # o(sig->tab), i(sig), ib(sig), z(tanh), d(exp+ln->tab)
                tmp = {}
                for g, func, dst in (
                    (2, AF.Sigmoid, 2),     # o -> tab slab 2
                    (0, AF.Sigmoid, None),  # i
                    (3, AF.Sigmoid, None),  # ibar
                    (1, AF.Tanh, None),     # z
                    (4, None, 3),           # d -> softplus -> tab slab 3
                ):
                    ps = p1ps.tile([128, 512], f32, tag="p1")
                    ps = ps[:, 0:cn]
                    for kt in range(KT):
                        nc.tensor.matmul(
                            ps, wt_sb[:, kt, g * 128:(g + 1) * 128],
                            et_sb[:, kt, ts], start=kt == 0, stop=kt == KT - 1)
                    if func is AF.Sigmoid and dst is not None:
                        nc.scalar.activation(
                            out=tab[:, dst, ts], in_=ps, func=func,
                            bias=bias_sb[:, g:g + 1])
                    elif func is not None:
                        t = p1t.tile([128, 512], f32, tag=f"t{g}")
                        t = t[:, 0:cn]
                        nc.scalar.activation(
                            out=t, in_=ps, func=func,
                            bias=bias_sb[:, g:g + 1])
                        tmp[g] = t
                    else:
                        # softplus(d) = Ln(1 + Exp(d))
                        t = p1t.tile([128, 512], f32, tag="tsp")
                        t = t[:, 0:cn]
                        nc.scalar.activation(
                            out=t, in_=ps, func=AF.Exp,
                            bias=bias_sb[:, g:g + 1])
                        nc.scalar.activation(
                            out=tab[:, 3, ts], in_=t, func=AF.Ln, bias=1.0)
                nc.vector.tensor_mul(
                    out=tab[:, 0, ts], in0=tmp[0][:], in1=tmp[1][:])
                nc.vector.tensor_mul(
                    out=tab[:, 1, ts], in0=tmp[3][:], in1=tmp[1][:])

            # ---- phase 2: expansion + pointwise, per wave ----------------------
            for w in range(NWAVES):
                js = slice(w * WAVE, (w + 1) * WAVE)
                ex = wpool.tile([128, 4, WAVE], f32, tag="ex")
                exc, excb, exgo, exgd = (ex[:, i, :] for i in range(4))

                # greedy cost-balanced piece assignment (ns estimates;
                # constants empirically best on HW)
                load = {"v": 4600.0, "s": 2300.0, "g": 0.0}
                plan = []
                for (t0, nt, reps, j0) in sorted(
                        pieces_by_wave[w], key=lambda p: -p[1] * p[2]):
                    cols = nt * reps
                    cost = {"v": 400 + cols * 2.9, "s": 350 + cols * 2.9,
                            "g": 1400 + cols * 12.0}
                    e = min(load, key=lambda k: load[k] + cost[k])
                    load[e] += cost[e]
                    plan.append((e, t0, nt, reps, j0))
                for e, t0, nt, reps, j0 in plan:
                    o0 = j0 - w * WAVE
                    src = tab[:, :, t0:t0 + nt].unsqueeze(3).broadcast_to(
                        [128, 4, nt, reps])
                    dst = ex[:, :, o0:o0 + nt * reps]
                    if e == "v":
                        nc.vector.tensor_copy(out=dst, in_=src)
                    elif e == "s":
                        nc.scalar.copy(out=dst, in_=src)
                    else:
                        nc.gpsimd.tensor_copy(out=dst, in_=src)

                for sl in range(4):
                    nc.sync.dma_start(out=out_d[1 + sl, :, js], in_=ex[:, sl, :])

                # dur broadcast via PE ones-outer-product, consumed from PSUM
                p = wpool.tile([128, WAVE], f32, tag="p")
                x = wpool.tile([128, WAVE], f32, tag="x")
                for h, eng in ((0, nc.vector), (1, nc.vector)):
                    ps = p2ps.tile([128, 512], f32, tag="p2")
                    hs = slice(h * 512, (h + 1) * 512)
                    nc.tensor.matmul(
                        ps[:], ones_sb[0:1, :],
                        dur_sb[0:1, w * WAVE + h * 512:w * WAVE + (h + 1) * 512],
                        start=True, stop=True)
                    eng.tensor_mul(out=p[:, hs], in0=exgd[:, hs], in1=ps[:])
                pb = wpool.tile([128, WAVE], bf16, tag="pb")
                xb = wpool.tile([128, WAVE], bf16, tag="xb")
                nc.scalar.activation(out=xb[:], in_=p[:], func=AF.Exp)
                nc.vector.tensor_sub(pb[:], exc, excb)
                nc.vector.tensor_mul(pb[:], pb[:], xb[:])
                nc.vector.tensor_add(p[:], pb[:], excb)
                nc.scalar.activation(out=x[:], in_=p[:], func=AF.Tanh)
                nc.vector.tensor_mul(out=p[:], in0=x[:], in1=exgo)

                nc.sync.dma_start(out=out_d[0, :, js], in_=p[:])

    nc.compile()
    return nc


def _marshal(event_seqs, duration_seqs, emb_table, W_rec, b_rec):
    import ml_dtypes

    ev = np.asarray(event_seqs)
    dur = np.asarray(duration_seqs, dtype=np.float32)
    emb = np.asarray(emb_table, dtype=np.float32)
    W = np.asarray(W_rec, dtype=np.float32)
    b = np.asarray(b_rec, dtype=np.float32)

    ev_tok = ev.T.reshape(-1)                      # token = t*B + b
    type_order, perm, pieces = _make_pieces(ev_tok)

    et = np.zeros((HIDDEN, TPAD), np.float32)
    et[:, :TYPES] = emb[type_order].T              # col t = NEW type id t
    et = et.astype(ml_dtypes.bfloat16)

    durneg = (-dur.T.reshape(-1)[perm]).reshape(1, NTOK).astype(ml_dtypes.bfloat16)

    in_maps = []
    for k in range(NCORES):
        wt = np.empty((HIDDEN, NGATES * 128), np.float32)
        bias = np.empty((128, NGATES), np.float32)
        for g, g7 in enumerate(GATE_ROWS):
            rows = slice(g7 * HIDDEN + 128 * k, g7 * HIDDEN + 128 * (k + 1))
            wt[:, g * 128:(g + 1) * 128] = W[rows, :HIDDEN].T
            bias[:, g] = b[rows]
        in_maps.append({
            "et": et, "wt": wt.astype(ml_dtypes.bfloat16),
            "bias": bias, "durneg": durneg,
        })
    return ev_tok, perm, pieces, in_maps


def _ensure_ntff_hook():
    import sys
    import types

    try:
        from antenv.axon_hooks import get_axon_ntff_profile_hook  # noqa: F401
        return
    except ImportError:
        pass
    try:
        import antenv
    except ImportError:
        return
    mod = types.ModuleType("antenv.axon_hooks")
    state = {"hook": None}
    mod.set_axon_ntff_profile_hook = lambda h: state.__setitem__("hook", h)
    mod.get_axon_ntff_profile_hook = lambda: state["hook"]
    sys.modules["antenv.axon_hooks"] = mod
    antenv.axon_hooks = mod
    try:
        from trn_agent_boot.trn_boot import _ntff_profile_via_ctypes

        hook = _ntff_profile_via_ctypes("/opt/axon/libaxon_pjrt.so")
        if hook is not None:
            mod.set_axon_ntff_profile_hook(hook)
    except Exception:
        pass


def kernel(event_seqs, duration_seqs, emb_table, W_rec, b_rec):
    global LAST_RESULTS, _CACHED
    from concourse.bass_utils import run_bass_kernel_spmd

    ev_tok, perm, pieces, in_maps = _marshal(
        event_seqs, duration_seqs, emb_table, W_rec, b_rec)

    key = hash(np.asarray(event_seqs).tobytes())
    if _CACHED is None or _CACHED[0] != key:
        _CACHED = (key, _build_nc(pieces))
    nc = _CACHED[1]

    trace = os.environ.get("KERNEL_TRACE", "") not in ("", "0")
    if trace:
        _ensure_ntff_hook()
    res = run_bass_kernel_spmd(nc, in_maps, list(range(NCORES)), trace=trace)
    LAST_RESULTS = res

    # ---- host-side output assembly (un-permute) ---------------------------
    srt = np.empty((5, NTOK, HIDDEN), np.float32)
    for k in range(NCORES):
        o = res.results[k]["out"]                  # [5, 128, NTOK]
        srt[:, :, 128 * k:128 * (k + 1)] = o.transpose(0, 2, 1)
    full = np.empty((5, NTOK, HIDDEN), np.float32)
    full[:, perm, :] = srt
    return full.reshape(5, T, B, HIDDEN)

